# revision 6
# baseline (speedup 1.0000x reference)
"""Trainium2 Bass kernel for nn_Attention_34471407518209.

The module computes (all 1x1 convs, BN in training mode):
    q    = Wq2 @ BN(Wq @ x + bq) + bq2
    k    = Wsr @ x + bsr
    attn = rowmax(q @ k^T)            # (B, C, 1)
    out  = Wc @ (attn * mean_c(x))    # outer product against channel-mean

Everything upstream of the rowmax is linear in x, so the whole computation
collapses onto per-batch Gram matrices G_b = x_b x_b^T (64x64), row sums
r_b, and channel means v_b:
    q = A x + c 1^T  with  A = Wq2 diag(g') Wq  (g' from BN stats, which are
    themselves functions of sum_b G_b and sum_b r_b)
    attn_b = [A|c] @ [[G_b, r_b],[r_b^T, N]] @ [Wsr|bsr]^T
    out_b  = (Wc @ rowmax(attn_b)) (x) v_b      # rank-1 outer product

Device phase 1 computes G_b - the only device pass over x.  The host
marshals x into an fp8-e4m3, transposed, K-tile-packed layout
    xp[pair, p, g, h, c] = x[pair, c, 256 g + 128 h + p]
so each 256-column group is ONE DoubleRow (double-pumped fp8) matmul
    psum[128, 128] += lhsT(xp[:, g])^T @ rhs(xp[:, g])
accumulating G for both batches of the pair directly in PSUM: no
on-device transposes at all.  Phase 1 is then DMA-read-bound at ~4.2 MB
per core (fp8) instead of PE-bound; fp8 Gram quantization contributes
~1.6e-3 max-rel error to the final output (tolerance 2e-2).

The row sums r_b and channel means v_b are computed exactly on the host
in fp32 (numpy reductions during marshalling - embedding a ones column
in xp for an on-device r trips an Ldweights ISA check, and the host
reductions are exact), and the tiny 65x65 stats/attn/rowmax math runs
on the host in fp64 between the two device phases.

Device phase 2 materializes the (B, C, N) rank-1 outer products
out_b = u_b v_b^T.  The host packs u with a K=2 interleaved block-diagonal
trick: lhsT[h, 2c+h] = u[c], so ONE matmul against rhs
[v[n0+w]; v[n0+hb/2+w]] (2, 512) fills all 128 psum partitions with
psum[2c+h, w] = u[c] * v[n0 + (hb/2)h + w].  Flattened partition-major
that IS the (c, h, w) element order of out[b, :, n0:n0+hb].  The staging
tiles and the output tensor are fp16 (the host upcasts to fp32), which
halves phase 2's HBM write traffic to 8 MB per core; fp16 staging adds
<6e-4 rel error.

Sharding: data-parallel over batch, 4 batches per core on 8 cores.
"""

import os
from contextlib import ExitStack

import numpy as np
import ml_dtypes

import concourse.bass as bass
import concourse.mybir as mybir
import concourse.tile as tile
from concourse import bacc
from concourse.bass_utils import run_bass_kernel_spmd

B, C, N = 32, 64, 16384
NCORES = 8
BPC = B // NCORES          # batches per core
PAIRS = BPC // 2           # batch pairs per core (2 batches share 128 partitions)
NPAIRS = B // 2            # total batch pairs
GROUPS = N // 256          # DoubleRow groups (256 n-columns each) per pair
EPS = 1e-5

FP8 = mybir.dt.float8e4
F16 = mybir.dt.float16
F32 = mybir.dt.float32
F32R = mybir.dt.float32r
DR = mybir.MatmulPerfMode.DoubleRow

NP_FP8 = ml_dtypes.float8_e4m3

_cache: dict = {}
LAST_RESULTS: dict = {}    # exec-time info for test harnesses


def _run(nc, in_maps, core_ids, trace):
    """run_bass_kernel_spmd with graceful fallback when the axon NTFF
    profiling hook is unavailable (chipless tunnel containers)."""
    if trace:
        try:
            return run_bass_kernel_spmd(nc, in_maps, core_ids, trace=True)
        except ModuleNotFoundError:
            os.environ["BASS_NEVER_TRACE"] = "1"
    return run_bass_kernel_spmd(nc, in_maps, core_ids)


def _build_phase1(rep=None) -> bass.Bass:
    """Per pair: one DoubleRow fp8 matmul per 256 n-columns accumulates
    gr[p] = G (128 x 128, both batches block-diagonal) in PSUM.
    DMA-read-bound."""
    nc = bacc.Bacc(trn_type="TRN2", target_bir_lowering=False)
    xp = nc.dram_tensor("xp", (PAIRS, 128, GROUPS, 2, 128), FP8,
                        kind="ExternalInput")
    gr = nc.dram_tensor("gr", (PAIRS, 128, 128), F32, kind="ExternalOutput")

    GS = int(os.environ.get("P1_GSLAB", "16"))      # groups per DMA slab
    n_slabs = GROUPS // GS

    with ExitStack() as ctx:
        tc = ctx.enter_context(tile.TileContext(nc))
        xpool = ctx.enter_context(tc.tile_pool(
            name="xslab", bufs=int(os.environ.get("P1_XBUFS", "4"))))
        grpsum = ctx.enter_context(tc.tile_pool(name="grpsum", bufs=2,
                                                space="PSUM"))
        opool = ctx.enter_context(tc.tile_pool(name="outs", bufs=2))

        def _body(_iv=None):
            for p in range(PAIRS):
                g_ps = grpsum.tile([128, 128], F32)
                for k in range(n_slabs):
                    xs = xpool.tile([128, GS, 2, 128], FP8)
                    nc.sync.dma_start(out=xs, in_=xp[p, :, k * GS:(k + 1) * GS])
                    for g in range(GS):
                        gg = k * GS + g
                        nc.tensor.matmul(
                            g_ps,
                            lhsT=xs[:, g, :, :],
                            rhs=xs[:, g, :, :],
                            start=(gg == 0),
                            stop=(gg == GROUPS - 1),
                            perf_mode=DR)
                gr_sb = opool.tile([128, 128], F32, tag="grsb")
                nc.vector.tensor_copy(out=gr_sb, in_=g_ps)
                nc.sync.dma_start(out=gr[p], in_=gr_sb)

        if rep is None:
            _body()
        else:
            with tc.For_i(0, rep, 1) as _iv:
                _body(_iv)

    nc.compile()
    return nc


def _build_phase2(rep=None) -> bass.Bass:
    nc = bacc.Bacc(trn_type="TRN2", target_bir_lowering=False)
    u = nc.dram_tensor("u", (2, BPC * 128), F32R, kind="ExternalInput")
    v = nc.dram_tensor("v", (BPC, N), F32R, kind="ExternalInput")
    out = nc.dram_tensor("out", (BPC, C, N), F16, kind="ExternalOutput")

    with ExitStack() as ctx:
        tc = ctx.enter_context(tile.TileContext(nc))
        upool = ctx.enter_context(tc.tile_pool(name="u", bufs=1))
        vpool = ctx.enter_context(tc.tile_pool(
            name="v", bufs=int(os.environ.get("P2_VBUFS", "3"))))
        opsum = ctx.enter_context(tc.tile_pool(name="opsum", bufs=8, space="PSUM"))
        obuf = ctx.enter_context(tc.tile_pool(
            name="obuf", bufs=int(os.environ.get("P2_OBUFS", "3"))))

        u_t = upool.tile([2, BPC * 128], F32R)
        nc.sync.dma_start(out=u_t, in_=u[:, :])

        HB = int(os.environ.get("P2_HB", "16384"))  # n-range per (v_t, ob) pair

        P2_ALT = os.environ.get("P2_ALT", "1") == "1"

        # v loads ride SWDGE so they never queue behind the output
        # writes in an HWDGE FIFO (HWDGE is FIFO per issuing engine)
        veng = nc.gpsimd if os.environ.get("P2_VSW", "1") == "1" else nc.sync

        def _group(b, u_b, n0, hb, gi):
            v_t = vpool.tile([2, hb // 2], F32R, tag="v_t")
            veng.dma_start(
                out=v_t,
                in_=v[b:b + 1, n0:n0 + hb].rearrange(
                    "one (r w) -> (one r) w", r=2),
            )
            ob = obuf.tile([128, hb // 2], F16, tag="ob")
            for t in range(hb // 1024):
                o_ps = opsum.tile([128, 512], F32)
                nc.tensor.matmul(
                    o_ps, lhsT=u_b,
                    rhs=v_t[:, t * 512:(t + 1) * 512],
                    start=True, stop=True,
                )
                if t % 2 == 0:
                    nc.vector.tensor_copy(out=ob[:, t * 512:(t + 1) * 512], in_=o_ps)
                else:
                    nc.scalar.copy(out=ob[:, t * 512:(t + 1) * 512], in_=o_ps)
            nways = int(os.environ.get("P2_NWAYS", "2")) if P2_ALT else 1
            deng = [nc.sync, nc.scalar, nc.gpsimd][gi % nways]
            deng.dma_start(out=out[b, :, n0:n0 + hb], in_=ob)

        def _body(_iv=None):
          gi = 0
          for b in range(BPC):
            u_b = u_t[:, b * 128:(b + 1) * 128]
            n0 = 0
            hbs = [HB] * (N // HB)
            if b == 0 and os.environ.get("P2_RAMP", "1") == "1":
                # shrink the first writes so the output DMA starts early
                hbs = [HB // 8, HB // 8, HB // 4, HB // 2] + hbs[1:]
            for hb in hbs:
                _group(b, u_b, n0, hb, gi)
                n0 += hb
                gi += 1

        if rep is None:
            _body()
        else:
            with tc.For_i(0, rep, 1) as _iv:
                _body(_iv)

    nc.compile()
    return nc


def _marshal_x(x: np.ndarray):
    """Returns (xp, r, v): xp (NPAIRS, 128, GROUPS, 2, 128) fp8 transposed
    K-tile-packed x, r = exact fp32 per-(batch,channel) row sums (B, C),
    v = exact fp32 per-batch channel means (B, N)."""
    x32 = np.ascontiguousarray(np.asarray(x, dtype=np.float32))
    v = x32.mean(axis=1)                                   # (B, N) fp32
    r = x32.sum(axis=2, dtype=np.float64)                  # (B, C) fp64
    x8 = x32.astype(NP_FP8)                                # (B, C, N)
    xr = x8.reshape(NPAIRS, 128, GROUPS, 2, 128)           # (pair,c,g,h,p)
    xp = np.ascontiguousarray(xr.transpose(0, 4, 2, 3, 1))  # (pair,p,g,h,c)
    return xp, r, v


def _host_math(G, r, Wq, bq, gamma, beta, Wq2, bq2, Wsr, bsr, Wc):
    """G: (B, C, C), r: (B, C) in fp64. Returns u: (B, C) fp64."""
    M = G.sum(axis=0) / (B * N)
    m = r.sum(axis=0) / (B * N)
    mu = Wq @ m + bq
    Eq2 = np.einsum("ij,jk,ik->i", Wq, M, Wq) + 2 * bq * (Wq @ m) + bq * bq
    var = Eq2 - mu * mu
    gp = gamma / np.sqrt(var + EPS)
    betap = beta - mu * gp
    A = Wq2 @ (gp[:, None] * Wq)
    c = Wq2 @ (gp * bq + betap) + bq2

    Aa = np.concatenate([A, c[:, None]], axis=1)            # (C, C+1)
    Wa = np.concatenate([Wsr, bsr[:, None]], axis=1)        # (C, C+1)
    u = np.zeros((B, C))
    for b in range(B):
        Ga = np.zeros((C + 1, C + 1))
        Ga[:C, :C] = G[b]
        Ga[:C, C] = r[b]
        Ga[C, :C] = r[b]
        Ga[C, C] = N
        attn = Aa @ Ga @ Wa.T
        u[b] = Wc @ attn.max(axis=1)
    return u


def kernel(x, Wq, bq, gamma, beta, Wq2, bq2, Wsr, bsr, Wc, H=None, W=None, **_):
    x = np.asarray(x)
    Wq = np.asarray(Wq, dtype=np.float64)
    bq = np.asarray(bq, dtype=np.float64)
    gamma = np.asarray(gamma, dtype=np.float64)
    beta = np.asarray(beta, dtype=np.float64)
    Wq2 = np.asarray(Wq2, dtype=np.float64)
    bq2 = np.asarray(bq2, dtype=np.float64)
    Wsr = np.asarray(Wsr, dtype=np.float64)
    bsr = np.asarray(bsr, dtype=np.float64)
    Wc = np.asarray(Wc, dtype=np.float64)

    if "p1" not in _cache:
        _cache["p1"] = _build_phase1()
        _cache["p2"] = _build_phase2()
    nc1, nc2 = _cache["p1"], _cache["p2"]

    trace = bool(os.environ.get("BASS_TRACE"))
    core_ids = list(range(NCORES))

    xp, r, v = _marshal_x(x)
    in_maps1 = [{"xp": xp[PAIRS * i: PAIRS * (i + 1)]} for i in range(NCORES)]
    res1 = _run(nc1, in_maps1, core_ids, trace)
    LAST_RESULTS["p1"] = res1

    # unpack per-core results: gr[p] = G over the 128 stacked channels
    G = np.zeros((B, C, C))
    for i in range(NCORES):
        gr_i = np.asarray(res1.results[i]["gr"], dtype=np.float64)
        for p in range(PAIRS):
            b0 = BPC * i + 2 * p
            G[b0] = gr_i[p, 0:64, 0:64]
            G[b0 + 1] = gr_i[p, 64:128, 64:128]

    u = _host_math(G, r, Wq, bq, gamma, beta, Wq2, bq2, Wsr, bsr, Wc)
    u = np.ascontiguousarray(u, dtype=np.float32)

    in_maps2 = []
    for i in range(NCORES):
        uc = u[BPC * i: BPC * (i + 1)]              # (BPC, 64)
        u2 = np.zeros((2, BPC * 128), dtype=np.float32)
        for b in range(BPC):
            u2[0, b * 128: (b + 1) * 128: 2] = uc[b]   # lhsT[0, 2c]   = u[c]
            u2[1, b * 128 + 1: (b + 1) * 128: 2] = uc[b]  # lhsT[1, 2c+1] = u[c]
        in_maps2.append({
            "u": u2,
            "v": np.ascontiguousarray(v[BPC * i: BPC * (i + 1)]),
        })
    res2 = _run(nc2, in_maps2, core_ids, trace)
    LAST_RESULTS["p2"] = res2

    out = np.empty((B, C, N), dtype=np.float32)
    for i in range(NCORES):
        out[BPC * i: BPC * (i + 1)] = res2.results[i]["out"].astype(np.float32)
    return out


# revision 40
# speedup vs baseline: 1.4071x; 1.4071x over previous
"""Trainium2 Bass kernel for nn_Attention_34471407518209.

The module computes (all 1x1 convs, BN in training mode):
    q    = Wq2 @ BN(Wq @ x + bq) + bq2
    k    = Wsr @ x + bsr
    attn = rowmax(q @ k^T)            # (B, C, 1)
    out  = Wc @ (attn * mean_c(x))    # outer product against channel-mean

Everything upstream of the rowmax is linear in x, so the whole computation
collapses onto per-batch Gram matrices G_b = x_b x_b^T (64x64), row sums
r_b, and channel means v_b:
    q = A x + c 1^T  with  A = Wq2 diag(g') Wq  (g' from BN stats, which are
    themselves functions of sum_b G_b and sum_b r_b)
    attn_b = [A|c] @ [[G_b, r_b],[r_b^T, N]] @ [Wsr|bsr]^T
    out_b  = (Wc @ rowmax(attn_b)) (x) v_b      # rank-1 outer product

Device phase 1 computes G_b - the only device pass over x.  The host
marshals x into an fp8-e4m3, transposed, K-tile-packed layout
    xp[pair, p, g, h, c] = x[pair, c, 256 g + 128 h + p]
so each 256-column group is ONE DoubleRow (double-pumped fp8) matmul
    psum[128, 128] += lhsT(xp[:, g])^T @ rhs(xp[:, g])
accumulating G for both batches of the pair directly in PSUM: no
on-device transposes at all.  Phase 1 reads ~4.2 MB per core (fp8, a
~14 us DMA floor measured by a DMA-only ablation) and runs at ~21 us
per iteration - the remainder is per-matmul dispatch/weight-load
overhead on the 128 DoubleRow matmuls, which large (32-group) slabs on
two alternating HWDGE queues minimize.  fp8 Gram quantization
contributes ~1.6e-3 max-rel error to the final output (tolerance 2e-2).

The row sums r_b and channel means v_b are computed exactly on the host
in fp32 (numpy reductions during marshalling - embedding a ones column
in xp for an on-device r trips an Ldweights ISA check, and the host
reductions are exact), and the tiny 65x65 stats/attn/rowmax math runs
on the host in fp64 between the two device phases.

Device phase 2 materializes the (B, C, N) rank-1 outer products
out_b = u_b v_b^T.  The host packs u with a K=2 interleaved block-diagonal
trick: lhsT[h, 2c+h] = u[c], so ONE matmul against rhs
[v[n0+w]; v[n0+hb/2+w]] (2, 512) fills all 128 psum partitions with
psum[2c+h, w] = u[c] * v[n0 + (hb/2)h + w].  Flattened partition-major
that IS the (c, h, w) element order of out[b, :, n0:n0+hb].  The staging
tiles and the output tensor are fp16 (the host upcasts to fp32), which
halves phase 2's HBM write traffic to 8 MB per core; fp16 staging adds
<6e-4 rel error.  Measured floors per For_i iteration: 64 dependency-free
512-col matmuls alone take ~23 us (per-instruction dispatch at the PE's
mid pstate), the 8 MB write ~26 us; the full pipeline lands at ~40-47 us
(window-dependent).  Ablations showed every added instruction (split
staging DMAs, telescoped tail groups, 2-bank psum copies, a third copy
engine, a ramp of small first groups) makes it slower - the phase is
sequencer/semaphore-dominated, so the minimal 1-group-per-batch
structure stands.

Sharding: data-parallel over batch, 4 batches per core on 8 cores.
"""

import os
from contextlib import ExitStack

import numpy as np
import ml_dtypes

import concourse.bass as bass
import concourse.mybir as mybir
import concourse.tile as tile
from concourse import bacc
from concourse.bass_utils import run_bass_kernel_spmd

B, C, N = 32, 64, 16384
NCORES = 8
BPC = B // NCORES          # batches per core
PAIRS = BPC // 2           # batch pairs per core (2 batches share 128 partitions)
NPAIRS = B // 2            # total batch pairs
GROUPS = N // 256          # DoubleRow groups (256 n-columns each) per pair
EPS = 1e-5

FP8 = mybir.dt.float8e4
F16 = mybir.dt.float16
F32 = mybir.dt.float32
F32R = mybir.dt.float32r
BF16 = mybir.dt.bfloat16
DR = mybir.MatmulPerfMode.DoubleRow

NP_FP8 = ml_dtypes.float8_e4m3

_cache: dict = {}
LAST_RESULTS: dict = {}    # exec-time info for test harnesses


def _run(nc, in_maps, core_ids, trace):
    """run_bass_kernel_spmd with graceful fallback when the axon NTFF
    profiling hook is unavailable (chipless tunnel containers)."""
    if trace:
        try:
            return run_bass_kernel_spmd(nc, in_maps, core_ids, trace=True)
        except ModuleNotFoundError:
            os.environ["BASS_NEVER_TRACE"] = "1"
    return run_bass_kernel_spmd(nc, in_maps, core_ids)


def _build_phase1(rep=None) -> bass.Bass:
    """Per pair: one DoubleRow fp8 matmul per 256 n-columns accumulates
    gr[p] = G (128 x 128, both batches block-diagonal) in PSUM.
    DMA-read-bound."""
    nc = bacc.Bacc(trn_type="TRN2", target_bir_lowering=False)
    xp = nc.dram_tensor("xp", (PAIRS, 128, GROUPS, 2, 128), FP8,
                        kind="ExternalInput")
    gr = nc.dram_tensor("gr", (PAIRS, 128, 128), F32, kind="ExternalOutput")

    GS = int(os.environ.get("P1_GSLAB", "32"))      # groups per DMA slab
    if os.environ.get("P1_TELE", "0") == "1":
        slab_sizes = [GS] * (GROUPS // GS - 1)
        rem = GROUPS - sum(slab_sizes)
        while rem > 2:
            slab_sizes.append(rem // 2)
            rem -= rem // 2
        slab_sizes.append(rem)                      # e.g. 16,16,16,8,4,2,2
    else:
        slab_sizes = [GS] * (GROUPS // GS)

    with ExitStack() as ctx:
        tc = ctx.enter_context(tile.TileContext(nc))
        xpool = ctx.enter_context(tc.tile_pool(
            name="xslab", bufs=int(os.environ.get("P1_XBUFS", "4"))))
        # (defaults GS=32/XBUFS=4/XALT=1 measured fastest: 21.3 us vs 23.8
        # for GS=16 single-queue; DMA-only floor is 14.0 us)
        grpsum = ctx.enter_context(tc.tile_pool(name="grpsum", bufs=2,
                                                space="PSUM"))
        opool = ctx.enter_context(tc.tile_pool(name="outs", bufs=2))

        NOMM = os.environ.get("P1_NOMM", "0") == "1"      # bench: DMA only
        XALT = os.environ.get("P1_XALT", "1") == "1"      # 2-queue x loads

        def _body(_iv=None):
            for p in range(PAIRS):
                g_ps = grpsum.tile([128, 128], F32)
                g0 = 0
                for k, gs in enumerate(slab_sizes):
                    xs = xpool.tile([128, gs, 2, 128], FP8, tag="xs")
                    xeng = nc.scalar if (XALT and (p * len(slab_sizes) + k) % 2) else nc.sync
                    xeng.dma_start(out=xs, in_=xp[p, :, g0:g0 + gs])
                    if not NOMM:
                        for g in range(gs):
                            gg = g0 + g
                            nc.tensor.matmul(
                                g_ps,
                                lhsT=xs[:, g, :, :],
                                rhs=xs[:, g, :, :],
                                start=(gg == 0),
                                stop=(gg == GROUPS - 1),
                                perf_mode=DR)
                    g0 += gs
                if NOMM:
                    continue
                gr_sb = opool.tile([128, 128], F32, tag="grsb")
                nc.vector.tensor_copy(out=gr_sb, in_=g_ps)
                nc.sync.dma_start(out=gr[p], in_=gr_sb)

        if rep is None:
            _body()
        else:
            with tc.For_i(0, rep, 1) as _iv:
                _body(_iv)

    nc.compile()
    return nc


def _build_phase2(rep=None) -> bass.Bass:
    nc = bacc.Bacc(trn_type="TRN2", target_bir_lowering=False)
    UVDT = BF16 if os.environ.get("P2_BF16", "0") == "1" else F32R
    u = nc.dram_tensor("u", (2, BPC * 128), UVDT, kind="ExternalInput")
    v = nc.dram_tensor("v", (BPC, N), UVDT, kind="ExternalInput")
    out = nc.dram_tensor("out", (BPC, C, N), F16, kind="ExternalOutput")

    with ExitStack() as ctx:
        tc = ctx.enter_context(tile.TileContext(nc))
        upool = ctx.enter_context(tc.tile_pool(name="u", bufs=1))
        vpool = ctx.enter_context(tc.tile_pool(
            name="v", bufs=int(os.environ.get("P2_VBUFS", "3"))))
        CP1024 = os.environ.get("P2_CP1024", "0") == "1"
        opsum = ctx.enter_context(tc.tile_pool(
            name="opsum", bufs=(4 if CP1024 else 8), space="PSUM"))
        obuf = ctx.enter_context(tc.tile_pool(
            name="obuf", bufs=int(os.environ.get("P2_OBUFS", "3"))))

        u_t = upool.tile([2, BPC * 128], UVDT)
        nc.sync.dma_start(out=u_t, in_=u[:, :])

        HB = int(os.environ.get("P2_HB", "16384"))  # n-range per (v_t, ob) pair

        # v loads ride SWDGE so they never queue behind the output
        # writes in an HWDGE FIFO (HWDGE is FIFO per issuing engine)
        veng = nc.gpsimd if os.environ.get("P2_VSW", "1") == "1" else nc.sync

        NODMA = os.environ.get("P2_NODMA", "0") == "1"    # bench: no out DMA
        NOCOPY = os.environ.get("P2_NOCOPY", "0") == "1"  # bench: matmul only
        ACT9 = os.environ.get("P2_ACT9", "0") == "1"      # 9:7 ACT:DVE copies

        def _copy(t, dst, src):
            if ACT9:
                use_act = (t % 16) % 2 == 0 or (t % 16) == 15
            else:
                use_act = t % 2 == 1
            if use_act:
                nc.scalar.copy(out=dst, in_=src)
            else:
                nc.vector.tensor_copy(out=dst, in_=src)

        dq = os.environ.get("P2_DQ", "sc")       # out-DMA queue cycle
        engs = {"s": nc.sync, "c": nc.scalar, "g": nc.gpsimd}
        SPLITW = int(os.environ.get("P2_SPLITW", "1"))  # out-DMA chunks/group

        def _group(b, u_b, n0, hb, gi):
            v_t = vpool.tile([2, hb // 2], UVDT, tag="v_t")
            veng.dma_start(
                out=v_t,
                in_=v[b:b + 1, n0:n0 + hb].rearrange(
                    "one (r w) -> (one r) w", r=2),
            )
            nsub = SPLITW if hb // 1024 >= 2 * SPLITW else 1
            wsub = hb // 2 // nsub                # staging cols per sub-tile
            obs = [obuf.tile([128, wsub], F16, tag=f"ob{s}", name=f"ob{s}")
                   for s in range(nsub)]
            write_out = (not NODMA) or gi == 0
            if NOCOPY and write_out:
                nc.vector.memset(obs[0], 0.0)
            oview = out[b, :, n0:n0 + hb].rearrange("c (h w) -> c h w", h=2)
            di = 0
            CP = 1024 if (CP1024 and hb >= 2048) else 512  # psum cols/copy
            for t in range(hb // (2 * CP)):
                o_ps = opsum.tile([128, CP], F32)
                for q in range(CP // 512):
                    nc.tensor.matmul(
                        o_ps[:, q * 512:(q + 1) * 512], lhsT=u_b,
                        rhs=v_t[:, (t * (CP // 512) + q) * 512:
                                (t * (CP // 512) + q + 1) * 512],
                        start=True, stop=True,
                    )
                s, w0 = divmod(t * CP, wsub)
                if not NOCOPY:
                    _copy(t, obs[s][:, w0:w0 + CP], o_ps)
                if write_out and w0 + CP == wsub and not NOCOPY:
                    deng = engs[dq[(gi + di) % len(dq)]]
                    deng.dma_start(out=oview[:, :, s * wsub:(s + 1) * wsub],
                                   in_=obs[s])
                    di += 1
            if write_out and NOCOPY:
                deng = engs[dq[gi % len(dq)]]
                deng.dma_start(out=oview[:, :, 0:wsub], in_=obs[0])

        FREE = os.environ.get("P2_FREE", "0") == "1"
        if FREE:
            # diagnostic: free-running matmuls, no cross-engine deps
            vconst = vpool.tile([2, 512], UVDT)
            nc.vector.memset(vconst, 0.5)

        def _body_free(_iv=None):
            o_ps = None
            for t in range(64):
                o_ps = opsum.tile([128, 512], F32)
                nc.tensor.matmul(o_ps, lhsT=u_t[:, 0:128], rhs=vconst,
                                 start=True, stop=True,
                                 skip_group_check=True)
            ob = obuf.tile([128, 512], F16, tag="ob")
            nc.vector.tensor_copy(out=ob, in_=o_ps)
            nc.sync.dma_start(out=out[0, :, 0:1024], in_=ob)

        def _body(_iv=None):
          if FREE:
            _body_free(_iv)
            return
          gi = 0
          for b in range(BPC):
            u_b = u_t[:, b * 128:(b + 1) * 128]
            n0 = 0
            hbs = [HB] * (N // HB)
            if b == 0 and os.environ.get("P2_RAMP", "0") == "1":
                # shrink the first writes so the output DMA starts early
                hbs = [HB // 8, HB // 8, HB // 4, HB // 2] + hbs[1:]
            tele = os.environ.get("P2_TELE", "0")
            if b == BPC - 1 and tele != "0":
                # telescope the final (un-overlappable) copy+write chain
                if tele == "1":
                    hbs = hbs[:-1] + [HB // 2, HB // 4, HB // 8, HB // 16,
                                      HB // 16]
                else:
                    hbs = hbs[:-1] + [HB // 2, HB // 4, HB // 4]
            for hb in hbs:
                _group(b, u_b, n0, hb, gi)
                n0 += hb
                gi += 1

        if rep is None:
            _body()
        else:
            with tc.For_i(0, rep, 1) as _iv:
                _body(_iv)

    nc.compile()
    return nc


def _marshal_x(x: np.ndarray):
    """Returns (xp, r, v): xp (NPAIRS, 128, GROUPS, 2, 128) fp8 transposed
    K-tile-packed x, r = exact fp32 per-(batch,channel) row sums (B, C),
    v = exact fp32 per-batch channel means (B, N)."""
    x32 = np.ascontiguousarray(np.asarray(x, dtype=np.float32))
    v = x32.mean(axis=1)                                   # (B, N) fp32
    r = x32.sum(axis=2, dtype=np.float64)                  # (B, C) fp64
    x8 = x32.astype(NP_FP8)                                # (B, C, N)
    xr = x8.reshape(NPAIRS, 128, GROUPS, 2, 128)           # (pair,c,g,h,p)
    xp = np.ascontiguousarray(xr.transpose(0, 4, 2, 3, 1))  # (pair,p,g,h,c)
    return xp, r, v


def _host_math(G, r, Wq, bq, gamma, beta, Wq2, bq2, Wsr, bsr, Wc):
    """G: (B, C, C), r: (B, C) in fp64. Returns u: (B, C) fp64."""
    M = G.sum(axis=0) / (B * N)
    m = r.sum(axis=0) / (B * N)
    mu = Wq @ m + bq
    Eq2 = np.einsum("ij,jk,ik->i", Wq, M, Wq) + 2 * bq * (Wq @ m) + bq * bq
    var = Eq2 - mu * mu
    gp = gamma / np.sqrt(var + EPS)
    betap = beta - mu * gp
    A = Wq2 @ (gp[:, None] * Wq)
    c = Wq2 @ (gp * bq + betap) + bq2

    Aa = np.concatenate([A, c[:, None]], axis=1)            # (C, C+1)
    Wa = np.concatenate([Wsr, bsr[:, None]], axis=1)        # (C, C+1)
    u = np.zeros((B, C))
    for b in range(B):
        Ga = np.zeros((C + 1, C + 1))
        Ga[:C, :C] = G[b]
        Ga[:C, C] = r[b]
        Ga[C, :C] = r[b]
        Ga[C, C] = N
        attn = Aa @ Ga @ Wa.T
        u[b] = Wc @ attn.max(axis=1)
    return u


def kernel(x, Wq, bq, gamma, beta, Wq2, bq2, Wsr, bsr, Wc, H=None, W=None, **_):
    x = np.asarray(x)
    Wq = np.asarray(Wq, dtype=np.float64)
    bq = np.asarray(bq, dtype=np.float64)
    gamma = np.asarray(gamma, dtype=np.float64)
    beta = np.asarray(beta, dtype=np.float64)
    Wq2 = np.asarray(Wq2, dtype=np.float64)
    bq2 = np.asarray(bq2, dtype=np.float64)
    Wsr = np.asarray(Wsr, dtype=np.float64)
    bsr = np.asarray(bsr, dtype=np.float64)
    Wc = np.asarray(Wc, dtype=np.float64)

    if "p1" not in _cache:
        _cache["p1"] = _build_phase1()
        _cache["p2"] = _build_phase2()
    nc1, nc2 = _cache["p1"], _cache["p2"]

    trace = bool(os.environ.get("BASS_TRACE"))
    core_ids = list(range(NCORES))

    xp, r, v = _marshal_x(x)
    in_maps1 = [{"xp": xp[PAIRS * i: PAIRS * (i + 1)]} for i in range(NCORES)]
    res1 = _run(nc1, in_maps1, core_ids, trace)
    LAST_RESULTS["p1"] = res1

    # unpack per-core results: gr[p] = G over the 128 stacked channels
    G = np.zeros((B, C, C))
    for i in range(NCORES):
        gr_i = np.asarray(res1.results[i]["gr"], dtype=np.float64)
        for p in range(PAIRS):
            b0 = BPC * i + 2 * p
            G[b0] = gr_i[p, 0:64, 0:64]
            G[b0 + 1] = gr_i[p, 64:128, 64:128]

    u = _host_math(G, r, Wq, bq, gamma, beta, Wq2, bq2, Wsr, bsr, Wc)
    u = np.ascontiguousarray(u, dtype=np.float32)

    np_uv = (ml_dtypes.bfloat16 if os.environ.get("P2_BF16", "0") == "1"
             else np.float32)
    in_maps2 = []
    for i in range(NCORES):
        uc = u[BPC * i: BPC * (i + 1)]              # (BPC, 64)
        u2 = np.zeros((2, BPC * 128), dtype=np.float32)
        for b in range(BPC):
            u2[0, b * 128: (b + 1) * 128: 2] = uc[b]   # lhsT[0, 2c]   = u[c]
            u2[1, b * 128 + 1: (b + 1) * 128: 2] = uc[b]  # lhsT[1, 2c+1] = u[c]
        in_maps2.append({
            "u": u2.astype(np_uv),
            "v": np.ascontiguousarray(v[BPC * i: BPC * (i + 1)]).astype(np_uv),
        })
    res2 = _run(nc2, in_maps2, core_ids, trace)
    LAST_RESULTS["p2"] = res2

    out = np.empty((B, C, N), dtype=np.float32)
    for i in range(NCORES):
        out[BPC * i: BPC * (i + 1)] = res2.results[i]["out"].astype(np.float32)
    return out
